# revision 13
# baseline (speedup 1.0000x reference)
"""GRU layer (Keras reset_after=True) on 8 Trainium2 NeuronCores.

B=64, T=1024, D=U=512. Returns final hidden state [64, 512].

v3: data-parallel over batch (8 rows/core, weights replicated), single
recurrence chain per core (the PE is ldweights-bound at ~32cyc/tile, so
splitting batch into pipelined chains only multiplies weight loads).

  - phase 1 (xm = x@W + b) is fused into the loop as PE filler during the
    gate windows, writing straight into PSUM (no DRAM roundtrip).
  - recurrence z/r/h matmuls write their own PSUM tiles; two DVE adds fold
    the phase-1 xm in (cross-matmul-group PSUM accumulation does not
    survive on hardware).
  - z columns of both weight matrices and biases are negated on host, so
    sigmoid yields zbar = 1-z directly: h' = h + zbar*(hc - h).
  - recurrent h-bias enters HH via rank-1 matmuls (ones vector), so no
    extra vector add for it.
  - h kept in f32 with a bf16 twin produced by a parallel DVE add (no
    scalar-engine copy on the critical path).
"""

import os
import sys

import numpy as np

if "/opt/trn_rl_repo" not in sys.path:
    sys.path.insert(0, "/opt/trn_rl_repo")
if "/root/.axon_site" not in sys.path:
    sys.path.insert(0, "/root/.axon_site")

import ml_dtypes  # noqa: E402

import concourse.bass as bass  # noqa: E402
import concourse.tile as tile  # noqa: E402
from concourse import mybir  # noqa: E402
from concourse.vector_clock import ScopedClock, VectorClock  # noqa: E402
import bass_rust as _bass_rust  # noqa: E402

BF16 = ml_dtypes.bfloat16

B, T, D = 64, 1024, 512
U = 512
NCORES = 8
BC = B // NCORES          # 8 batch rows per core
KC = U // 128             # 4 k-chunks
MC = 3 * U // 128         # 12 m-chunks
UNROLL = 16               # steps per hw-loop iteration
TB = 8                    # steps per phase-1 block (N = TB*BC = 64)
NBLK = UNROLL // TB       # block slots (2)
NB = TB * BC              # phase-1 matmul N (64)
TPAD = 32                 # zero-padded trailing steps for prefetch overrun

# ---------------------------------------------------------------------------
# Workaround: walrus in this container rejects >1 sync-wait command on the
# final Tile drain. Split the global-clock waits across SP nops.
def _patched_drain_and_barrier(self, tick_clock, wait_clock):
    nc = self.nc
    gc = tick_clock.global_clock
    n = len(gc)
    procs = [i for i in range(n) if gc.peek_next(i) - 1 > 0]
    for p in procs:
        vec = [0] * n
        vec[p] = gc.peek_next(p) - 1
        nop_inst = nc.sync.nop(nofuse=True, hint="drain_split")
        wait_clock.add_sem_waits(nop_inst.ins, ScopedClock({None: VectorClock(vec)}))
    nc.sync.drain()
    nc.all_engine_barrier()
    assert self.sems is not None
    popped = nc._tile_sem_poison_stack.pop()
    assert popped is self._sem_poison
    nc.clear_and_free_semaphores(list(self.sems.allocated().values()))
    nc.all_engine_barrier()


tile.TileContext._drain_and_barrier = _patched_drain_and_barrier


def _split_waits(nc, maxw=1):
    """Walrus here only accepts `maxw` sync-wait commands per instruction.
    Move excess waits onto same-engine NoOps inserted just before."""
    nsplit = 0
    for f in nc.m.functions:
        for bb in f.blocks:
            insts = bb.instructions
            i = 0
            while i < len(insts):
                inst = insts[i]
                si = inst.sync_info
                if si is not None and si.on_wait and len(si.on_wait) > maxw:
                    waits = list(si.on_wait)
                    keep = waits[-maxw:]
                    extra = waits[:-maxw]
                    si.on_wait = keep
                    for k, w in enumerate(extra):
                        nop = mybir.InstNoOp(
                            name=f"{inst.name}-wsplit{k}",
                            opcode="NoOp",
                            engine=inst.engine,
                            debug=inst.debug,
                            ins=[],
                            outs=[],
                            sync_info=mybir.SyncInfo(on_wait=[w], on_update=[]),
                        )
                        insts.insert(i, nop)
                        nc.register_instruction(nop, overwrite=True)
                        i += 1
                        nsplit += 1
                i += 1
    return nsplit


# NTFF profiling hook (image lacks the boot-time wiring).
if os.environ.get("TRN_TERMINAL_POOL_IPS"):
    try:
        from antenv.axon_hooks import set_axon_ntff_profile_hook
        from trn_agent_boot.trn_boot import _ntff_profile_via_ctypes

        _h = _ntff_profile_via_ctypes("/opt/axon/libaxon_pjrt.so")
        if _h is not None:
            set_axon_ntff_profile_hook(_h)
    except Exception:
        pass

# ---------------------------------------------------------------------------
_NC = None

XCOLS = (T + TPAD) * BC   # x columns incl. prefetch pad


def _build_nc():
    f32 = mybir.dt.float32
    bf16 = mybir.dt.bfloat16
    nc = bass.Bass(target_bir_lowering=False)

    # x already transposed: x_bf[k, p, t*BC + b]
    x_bf = nc.dram_tensor("x_bf", [KC, 128, XCOLS], bf16, kind="ExternalInput")
    kern_in = nc.dram_tensor("kern_bf", [D, 3 * U], bf16, kind="ExternalInput")
    rker_in = nc.dram_tensor("rker_bf", [U, 3 * U], bf16, kind="ExternalInput")
    btot_in = nc.dram_tensor("btot", [1, 3 * U], bf16, kind="ExternalInput")
    brh_in = nc.dram_tensor("brh", [1, U], bf16, kind="ExternalInput")
    ident_in = nc.dram_tensor("ident", [128, 128], bf16, kind="ExternalInput")
    hT_out = nc.dram_tensor("hT_out", [128, KC, BC], f32, kind="ExternalOutput")

    Sig = mybir.ActivationFunctionType.Sigmoid
    Tanh = mybir.ActivationFunctionType.Tanh
    ET = mybir.EngineType

    with tile.TileContext(nc) as tc:
        with (
            tc.tile_pool(name="singles", bufs=1) as singles,
            tc.tile_pool(name="p1", bufs=1, space="PSUM") as p1pool,
            tc.tile_pool(name="gp", bufs=1, space="PSUM") as gppool,
        ):
            # ---- constants into SBUF -------------------------------------
            kern_sb = singles.tile([128, KC, MC, 128], bf16, tag="kern")
            nc.sync.dma_start(
                out=kern_sb,
                in_=kern_in.rearrange("(k p) (m c) -> p k m c", p=128, c=128),
            )
            R_sb = singles.tile([128, KC, MC, 128], bf16, tag="rker")
            nc.sync.dma_start(
                out=R_sb,
                in_=rker_in.rearrange("(k p) (m c) -> p k m c", p=128, c=128),
            )
            btot_sb = singles.tile([1, 3 * U], bf16, tag="btot")
            nc.sync.dma_start(out=btot_sb, in_=btot_in[:, :])
            brh_sb = singles.tile([1, U], bf16, tag="brh")
            nc.sync.dma_start(out=brh_sb, in_=brh_in[:, :])
            ident_sb = singles.tile([128, 128], bf16, tag="ident")
            nc.sync.dma_start(out=ident_sb, in_=ident_in[:, :])
            ones_sb = singles.tile([1, NB], bf16, tag="ones")
            nc.vector.memset(ones_sb, 1.0)

            # ---- persistent state ----------------------------------------
            h32 = [
                singles.tile([128, KC, BC], f32, tag=f"h32_{p}", name=f"h32_{p}")
                for p in range(2)
            ]
            hbf = [
                singles.tile([128, KC, BC], bf16, tag=f"hbf_{p}", name=f"hbf_{p}")
                for p in range(2)
            ]
            nc.vector.memset(h32[0], 0.0)
            nc.vector.memset(hbf[0], 0.0)

            # bf16 SBUF copy of the phase-1 z|r block (identity-matmul rhs;
            # the scalar engine casts f32->bf16 during the block copy)
            p1zr_sb = [
                singles.tile([128, 8, NB], bf16, tag=f"p1zrs_{s}", name=f"p1zrs_{s}")
                for s in range(NBLK)
            ]

            # x staging (one tile per block slot, rewritten per iteration)
            xt = [
                singles.tile([128, KC, NB], bf16, tag=f"xt_{s}", name=f"xt_{s}")
                for s in range(NBLK)
            ]

            # phase-1 PSUM: z|r in one bank-sized tile, h in another, per slot
            p1zr = [
                p1pool.tile([128, 8, NB], f32, tag=f"p1zr_{s}", name=f"p1zr_{s}")
                for s in range(NBLK)
            ]
            p1h = [
                p1pool.tile([128, 4, NB], f32, tag=f"p1h_{s}", name=f"p1h_{s}")
                for s in range(NBLK)
            ]
            # recurrence PSUM (per step parity)
            zr = [
                gppool.tile([128, 8, BC], f32, tag=f"zr_{p}", name=f"zr_{p}")
                for p in range(2)
            ]
            hh = [
                gppool.tile([128, 4, BC], f32, tag=f"hh_{p}", name=f"hh_{p}")
                for p in range(2)
            ]
            # SBUF gate temps (per step parity)
            wk = [
                {
                    n: singles.tile(
                        [128, 4, BC], f32, tag=f"{n}_{p}", name=f"{n}_{p}"
                    )
                    for n in ("tr", "tz", "rs", "zb", "t3", "t4", "hc", "dd", "e2")
                }
                for p in range(2)
            ]

            def ph1_block(s):
                """Phase-1 GEMM granules writing slot s (reads xt[s])."""
                gran = []
                for m in range(MC):
                    def emit(m=m):
                        dst = p1zr[s][:, m, :] if m < 8 else p1h[s][:, m - 8, :]
                        for k in range(KC):
                            nc.tensor.matmul(
                                dst,
                                lhsT=kern_sb[:, k, m, :],
                                rhs=xt[s][:, k, :],
                                start=(k == 0),
                                stop=False,
                            )
                        nc.tensor.matmul(
                            dst,
                            lhsT=btot_sb[0:1, m * 128 : (m + 1) * 128],
                            rhs=ones_sb,
                            start=False,
                            stop=True,
                        )

                    gran.append(emit)

                def copy_zr():
                    nc.scalar.copy(p1zr_sb[s], p1zr[s])

                gran.append(copy_zr)
                return gran

            def step(j, slot, js, ph1_gran):
                """One recurrence step. j: parity index; slot: phase-1 block
                slot; js: step offset in block; ph1_gran: filler granules."""
                par = j % 2
                nxt = 1 - par
                hbI = hbf[par]
                h32I = h32[par]
                ZR = zr[par]
                HH = hh[par]
                W = wk[par]
                s0 = js * BC

                # hh bias preload via rank-1 (no dependency on h)
                for m in range(4):
                    nc.tensor.matmul(
                        HH[:, m, :],
                        lhsT=brh_sb[0:1, m * 128 : (m + 1) * 128],
                        rhs=ones_sb[0:1, 0:BC],
                        start=True,
                        stop=False,
                    )
                # r-gate: identity matmul seeds ZR with xm_r, R-matmuls
                # accumulate hm_r on top within the same PSUM group
                nc.tensor.matmul(
                    ZR[:, 4:8, :],
                    lhsT=ident_sb,
                    rhs=p1zr_sb[slot][:, 4:8, s0 : s0 + BC],
                    start=True,
                    stop=False,
                )
                for m in range(4):
                    for k in range(KC):
                        nc.tensor.matmul(
                            ZR[:, m + 4, :],
                            lhsT=R_sb[:, k, m + 4, :],
                            rhs=hbI[:, k, :],
                            start=False,
                            stop=(m == 3 and k == KC - 1),
                            skip_group_check=True,
                        )
                nc.scalar.activation(W["rs"], ZR[:, 4:8, :], Sig)

                # hh matmuls
                for m in range(4):
                    for k in range(KC):
                        nc.tensor.matmul(
                            HH[:, m, :],
                            lhsT=R_sb[:, k, m + 8, :],
                            rhs=hbI[:, k, :],
                            start=False,
                            stop=(k == KC - 1),
                        )
                nc.vector.tensor_mul(W["t3"], W["rs"], HH)

                # z-gate (negated -> sigmoid gives 1-z), identity-seeded
                nc.tensor.matmul(
                    ZR[:, 0:4, :],
                    lhsT=ident_sb,
                    rhs=p1zr_sb[slot][:, 0:4, s0 : s0 + BC],
                    start=True,
                    stop=False,
                )
                for m in range(4):
                    for k in range(KC):
                        nc.tensor.matmul(
                            ZR[:, m, :],
                            lhsT=R_sb[:, k, m, :],
                            rhs=hbI[:, k, :],
                            start=False,
                            stop=(m == 3 and k == KC - 1),
                            skip_group_check=True,
                        )

                nc.vector.tensor_add(
                    W["t4"], W["t3"], p1h[slot][:, :, s0 : s0 + BC]
                )
                hci = nc.scalar.activation(W["hc"], W["t4"], Tanh)
                zbi = nc.scalar.activation(W["zb"], ZR[:, 0:4, :], Sig)
                zbi.ins.add_nosync_dependencies_from(
                    _bass_rust.InstructionNameOrderedSet([hci.ins.name])
                )

                nc.vector.tensor_sub(W["dd"], W["hc"], h32I)
                nc.vector.tensor_mul(W["e2"], W["zb"], W["dd"])
                nc.vector.tensor_add(hbf[nxt], W["e2"], h32I)
                nc.vector.tensor_add(h32[nxt], W["e2"], h32I)

                # drip phase-1 filler into the PE stream
                while ph1_gran:
                    ph1_gran.pop(0)()

            # ---- preamble -------------------------------------------------
            for k in range(KC):
                nc.sync.dma_start(
                    out=xt[0][:, k, :], in_=x_bf[k, :, 0:NB]
                )
            for g in ph1_block(0):
                g()
            for k in range(KC):
                nc.sync.dma_start(
                    out=xt[1][:, k, :], in_=x_bf[k, :, NB : 2 * NB]
                )
            for k in range(KC):
                nc.sync.dma_start(
                    out=xt[0][:, k, :], in_=x_bf[k, :, 2 * NB : 3 * NB]
                )

            # ---- main loop -----------------------------------------------
            # iv counts x columns (BC per step). Steps j=0..TB-1 read slot 0,
            # j=TB..2TB-1 read slot 1. Slot-0 granules (next iteration's
            # data) drip during j=TB.., slot-1 granules at body end filling
            # the last gate window. x DMA for a slot follows its granules.
            with tc.For_i(
                0,
                T * BC,
                UNROLL * BC,
                hint_engines=(ET.PE, ET.DVE, ET.Activation, ET.SP),
            ) as iv:
                # slot-1 granules compute THIS body's second-half data and
                # drip during j=0..TB-1 (their readers are j=TB..2TB-1);
                # slot-0 granules compute the NEXT body's first half and
                # drip during j=TB..2TB-1. DMAs spread mid-body.
                gran1 = ph1_block(1)
                gran0 = ph1_block(0)
                per = (len(gran0) + TB - 1) // TB

                for j in range(UNROLL):
                    slot = j // TB
                    js = j % TB
                    if j < TB:
                        take = gran1[:per]
                        gran1 = gran1[per:]
                    else:
                        take = gran0[:per]
                        gran0 = gran0[per:]
                    step(j, slot, js, take)
                    if j == TB + 2:
                        # xt[1] free once gran1 is done (end of j=TB-1)
                        for k in range(KC):
                            nc.sync.dma_start(
                                out=xt[1][:, k, :],
                                in_=x_bf[
                                    k, :,
                                    bass.ds(iv + (UNROLL + TB) * BC, NB),
                                ],
                            )
                    if j == UNROLL - 1:
                        for k in range(KC):
                            nc.sync.dma_start(
                                out=xt[0][:, k, :],
                                in_=x_bf[
                                    k, :, bass.ds(iv + 2 * UNROLL * BC, NB)
                                ],
                            )

            # ---- output --------------------------------------------------
            nc.sync.dma_start(out=hT_out[:, :, :], in_=h32[0])

    _split_waits(nc, maxw=1)
    return nc


def kernel(x, kernel, recurrent_kernel, bias):
    global _NC
    from concourse.bass_utils import run_bass_kernel_spmd

    x = np.ascontiguousarray(np.asarray(x, dtype=np.float32))
    kern = np.asarray(kernel, dtype=np.float32)
    rker = np.asarray(recurrent_kernel, dtype=np.float32)
    bias = np.asarray(bias, dtype=np.float32)

    if _NC is None:
        _NC = _build_nc()
    nc = _NC

    # negate z columns so sigmoid yields zbar = 1-z
    kern2 = kern.copy()
    kern2[:, :U] = -kern2[:, :U]
    rker2 = rker.copy()
    rker2[:, :U] = -rker2[:, :U]
    btot = bias[0] + np.concatenate([bias[1][: 2 * U], np.zeros(U, np.float32)])
    btot[:U] = -btot[:U]

    kern_bf = np.ascontiguousarray(kern2.astype(BF16))
    rker_bf = np.ascontiguousarray(rker2.astype(BF16))
    btot_bf = np.ascontiguousarray(btot.reshape(1, 3 * U).astype(BF16))
    brh_bf = np.ascontiguousarray(bias[1][2 * U :].reshape(1, U).astype(BF16))
    ident = np.ascontiguousarray(np.eye(128, dtype=BF16))

    # pre-transpose per core: x_t[k, p, t*BC + b] = x[row, t, k*128 + p]
    xt_all = np.zeros((NCORES, KC, 128, XCOLS), dtype=BF16)
    xr = (
        x.reshape(NCORES, BC, T, KC, 128)
        .transpose(0, 3, 4, 2, 1)
        .reshape(NCORES, KC, 128, T * BC)
        .astype(BF16)
    )
    xt_all[:, :, :, : T * BC] = xr

    in_maps = []
    for core in range(NCORES):
        in_maps.append(
            {
                "x_bf": np.ascontiguousarray(xt_all[core]),
                "kern_bf": kern_bf,
                "rker_bf": rker_bf,
                "btot": btot_bf,
                "brh": brh_bf,
                "ident": ident,
            }
        )

    trace = bool(int(os.environ.get("GRU_TRACE", "0")))
    kw = {}
    if trace:
        kw = dict(
            trace=True,
            trace_cores=[0],
            tmpdir=os.environ.get("GRU_TRACE_DIR", "/root/problem/work/trace_gru"),
        )
    res = run_bass_kernel_spmd(nc, in_maps, core_ids=list(range(NCORES)), **kw)
    if trace:
        print("HW exec time:", res.exec_time_ns, "ns")

    out = np.empty((B, U), np.float32)
    for core in range(NCORES):
        hT = res.results[core]["hT_out"].reshape(128, KC, BC)
        out[core * BC : (core + 1) * BC] = hT.transpose(2, 1, 0).reshape(BC, U)
    return out
